# revision 19
# baseline (speedup 1.0000x reference)
"""Multi-head causal attention (B=4, T=1024, C=1024, H=16, D=64) on 8 TRN2 cores.

Sharding: tensor-parallel over heads. Core i owns heads {2i, 2i+1}:
  - x is replicated (sent pre-transposed as xT [C, B*T], bf16)
  - Wq/Wk/Wv sharded over heads -> per-core [C, 128] (2 heads concat on D)
  - row-parallel output projection: per-core Wp rows [128, C]; host sums the
    8 partial [B*T, C] outputs (the all-reduce) and adds bp.

v2 scheduling rewrite (vs 175us baseline): the PE stream is kept gap-free so
the HAM clock gate stays at 2.4 GHz:
  - warm-up junk matmuls cover the input-DMA lead-in
  - q,k matmuls first, scores s=0 early, v/transposes + prev-batch proj
    interleaved into the scores phase so ACT exp has a head start
  - proj halves run from a pending-queue two slots behind, so the final
    normalize (DMA-bounce reciprocal) hides under real PE work
  - elementwise ops balanced: ACT=exp+den, DVE=qk copies/masks/normalize,
    GPSIMD=v copies/vaug, proj copies split DVE/ACT/GPSIMD
"""

import ml_dtypes
import numpy as np

B, T, C = 4, 1024, 1024
H, D = 16, 64
NCORES = 8
HPC = H // NCORES      # heads per core = 2
D2 = HPC * D           # 128
BT = B * T
SCALE = 1.0 / np.sqrt(np.float32(C))  # 1/32
BF16 = ml_dtypes.bfloat16

_compiled = None

NWARM = 16


def _split_multi_waits(nc, mybir, maxw=1):
    """Walrus in this container encodes at most one sync wait per
    instruction (fp32 self-loading matmuls and drains overflow).  Hoist
    excess waits onto same-engine NoOps inserted just before."""
    for fn in nc.m.functions:
        for bb in fn.blocks:
            new = []
            for inst in bb.instructions:
                si = inst.sync_info
                waits = list(si.on_wait) if (si is not None and si.on_wait) else []
                if len(waits) > maxw:
                    extra, keep = waits[:-maxw], waits[-maxw:]
                    for j, w in enumerate(extra):
                        new.append(
                            mybir.InstNoOp(
                                name=f"{inst.name}-wsplit{j}",
                                engine=inst.engine,
                                sync_info=mybir.SyncInfo(on_wait=[w], on_update=[]),
                                bass_nofuse=True,
                            )
                        )
                    inst.sync_info = mybir.SyncInfo(
                        on_wait=keep,
                        on_update=list(si.on_update) if si.on_update else [],
                    )
                new.append(inst)
            bb.instructions = new


def _build():
    import concourse.bass as bass
    import concourse.mybir as mybir
    import concourse.tile as tile

    f32 = mybir.dt.float32
    bf = mybir.dt.bfloat16
    EXP = mybir.ActivationFunctionType.Exp

    nc = bass.Bass("TRN2", target_bir_lowering=False, debug=False, num_devices=NCORES)

    xT_d = nc.dram_tensor("xT", [C, BT], bf, kind="ExternalInput").ap()
    # host pre-shuffles each weight to [p, k, m] so the DMA is contiguous
    wq_d = nc.dram_tensor("wq", [128, C // 128, D2], bf, kind="ExternalInput").ap()
    wk_d = nc.dram_tensor("wk", [128, C // 128, D2], bf, kind="ExternalInput").ap()
    wv_d = nc.dram_tensor("wv", [128, C // 128, D2], bf, kind="ExternalInput").ap()
    wp_d = nc.dram_tensor("wp", [D2, C], bf, kind="ExternalInput").ap()
    mask_d = nc.dram_tensor("mask", [128, 128], bf, kind="ExternalInput").ap()
    ident_d = nc.dram_tensor("ident", [128, 128], bf, kind="ExternalInput").ap()
    out_d = nc.dram_tensor("out", [BT, C], bf, kind="ExternalOutput").ap()

    KC = C // 128  # 8 contraction chunks over C
    NS = T // 128  # 8 s-chunks
    NH = 2         # two 512-wide t halves

    import concourse.bass as _bass

    with tile.TileContext(nc) as tc:
        with (
            tc.tile_pool(name="const", bufs=1) as constp,
            tc.tile_pool(name="xin", bufs=1) as xinp,
            tc.tile_pool(name="qkv", bufs=2) as qkvp,
            tc.tile_pool(name="vaug", bufs=2) as vaugp,
            tc.tile_pool(name="exps", bufs=18) as expp,
            tc.tile_pool(name="smalls", bufs=4) as smallp,
            tc.tile_pool(name="outt", bufs=3) as outtp,
            tc.tile_pool(name="pout", bufs=4) as poutp,
            tc.tile_pool(name="dram", bufs=2, space="DRAM") as dramp,
            tc.tile_pool(name="ps512", bufs=2, space="PSUM") as ps512,
            tc.tile_pool(name="psatt", bufs=2, space="PSUM") as psatt,
            tc.tile_pool(name="psvt", bufs=2, space="PSUM") as psvt,
            tc.tile_pool(name="psproj", bufs=2, space="PSUM") as psproj,
        ):
            # ---- constants / warmup ----
            wq_s = constp.tile([128, KC, D2], bf, tag="wq")
            wk_s = constp.tile([128, KC, D2], bf, tag="wk")
            wv_s = constp.tile([128, KC, D2], bf, tag="wv")
            wp_s = constp.tile([128, C], bf, tag="wp")
            mask_s = constp.tile([128, 128], bf, tag="mask")
            ident = constp.tile([128, 128], bf, tag="ident")
            junk = constp.tile([128, 512], bf, tag="junk")

            # Warm-up: junk matmuls with no DMA deps fill the PE stream while
            # inputs land, so HAM un-throttles before real work and never
            # re-throttles (any later stall is << the 3.4us MID window).
            nc.vector.memset(junk[:], 0.0)
            for i in range(NWARM):
                pw = ps512.tile([128, 512], f32, tag="ps512", name=f"warm{i}")
                nc.tensor.matmul(pw[:], junk[:, 0:128], junk[:], start=True, stop=True)

            # Input DMAs, critical-first: wq + the first x chunk gate the
            # first real matmul.  x is loaded in one shot for all batches
            # (k-chunk DMAs of contiguous 8KB rows - 128 descriptors each);
            # weights arrive pre-shuffled from the host so each is a single
            # contiguous [128, 1024] transfer.
            xba = xinp.tile([128, KC, BT], bf, tag="xba", name="xba", bufs=1)
            nc.sync.dma_start(wq_s[:], wq_d)
            nc.sync.dma_start(xba[:, 0, :], xT_d[0:128, :])
            nc.sync.dma_start(wk_s[:], wk_d)
            nc.sync.dma_start(wv_s[:], wv_d)
            for k in range(1, KC):
                nc.sync.dma_start(
                    xba[:, k, :], xT_d[k * 128:(k + 1) * 128, :]
                )
            nc.sync.dma_start(mask_s[:], mask_d)
            nc.sync.dma_start(ident[:], ident_d)
            nc.sync.dma_start(wp_s[:], wp_d)

            def emit_qk(b, xb):
                qT = qkvp.tile([128, T], bf, tag="qT", name=f"qT{b}")
                kT = qkvp.tile([128, T], bf, tag="kT", name=f"kT{b}")
                for w_s, oT in ((wq_s, qT), (wk_s, kT)):
                    for half in range(NH):
                        ps = ps512.tile([128, 512], f32, tag="ps512")
                        for k in range(KC):
                            nc.tensor.matmul(
                                ps[:],
                                w_s[:, k, :],
                                xb[:, k, half * 512:(half + 1) * 512],
                                start=(k == 0),
                                stop=(k == KC - 1),
                            )
                        nc.vector.tensor_copy(
                            oT[:, half * 512:(half + 1) * 512], ps[:]
                        )
                return qT, kT

            def emit_v_mms(b, xb, half, vT):
                ps = ps512.tile([128, 512], f32, tag="ps512")
                for k in range(KC):
                    nc.tensor.matmul(
                        ps[:],
                        wv_s[:, k, :],
                        xb[:, k, half * 512:(half + 1) * 512],
                        start=(k == 0),
                        stop=(k == KC - 1),
                    )
                nc.scalar.copy(vT[:, half * 512:(half + 1) * 512], ps[:])

            def emit_vtrans(b, vT, vaug, srange):
                for s in srange:
                    pv = psvt.tile([128, 128], bf, tag="psvt")
                    nc.tensor.transpose(
                        pv[:], vT[:, s * 128:(s + 1) * 128], ident[:]
                    )
                    nc.vector.tensor_copy(vaug[:, s, :, 0:64], pv[:])

            # Scores psums alternate between the two pools: transposes are
            # idle during the s-loop, so this gives a 4-bank rotation and the
            # ACT exp backlog ~1.5us of elasticity before PE stalls on a bank.
            _sctr = [0]

            def score_ps():
                _sctr[0] += 1
                pool = ps512 if _sctr[0] % 2 else psvt
                tag = "ps512" if _sctr[0] % 2 else "psvt"
                return pool.tile(
                    [128, 512], f32, tag=tag, name=f"sc{_sctr[0]}"
                )

            def emit_scores_s(b, s, qT, kT, exs):
                s0 = s * 128
                d1 = max(0, s0 - 512)
                for h in range(HPC):
                    hp = slice(h * 64, (h + 1) * 64)
                    ex = expp.tile(
                        [128, 1024], bf, tag="ex", bufs=18, name=f"ex{b}_{h}_{s}"
                    )
                    exs[(h, s)] = ex
                    if s < 4:  # t-half0 piece: cols [s0, 512)
                        w0 = 512 - s0
                        pa = score_ps()
                        nc.tensor.matmul(
                            pa[:, 0:w0],
                            kT[hp, s0:s0 + 128],
                            qT[hp, s0:512],
                            start=True,
                            stop=True,
                        )
                        nc.scalar.activation(
                            ex[:, 0:w0], pa[:, 0:w0], EXP, scale=float(SCALE)
                        )
                    # t-half1 piece: cols [max(512, s0), 1024)
                    w1 = 512 - d1
                    pb = score_ps()
                    nc.tensor.matmul(
                        pb[:, 0:w1],
                        kT[hp, s0:s0 + 128],
                        qT[hp, 512 + d1:T],
                        start=True,
                        stop=True,
                    )
                    nc.scalar.activation(
                        ex[:, 512 - s0 + d1:T - s0],
                        pb[:, 0:w1],
                        EXP,
                        scale=float(SCALE),
                    )
                    nc.gpsimd.tensor_mul(ex[:, 0:128], ex[:, 0:128], mask_s[:])

            def emit_po0_s(b, s, vaug, exs, po0):
                assert 0 <= s <= 3
                s0 = s * 128
                for h in range(HPC):
                    nc.tensor.matmul(
                        po0[h][0:65, s0:512],
                        vaug[:, s, h, 0:65],
                        exs[(h, s)][:, 0:512 - s0],
                        start=(s == 0),
                        stop=(s == 3),
                    )

            def emit_normalize_half(b, half, po_h, outT2):
                t0 = half * 512
                den2 = smallp.tile(
                    [1, 2 * 512], f32, tag="den2", bufs=2, name=f"den2_{b}_{half}"
                )
                for h in range(HPC):
                    nc.scalar.copy(
                        den2[0:1, h * 512:(h + 1) * 512], po_h[h][64:65, 0:512]
                    )
                # Packed reciprocal: bounce the 1024 denominators through DRAM
                # to use all 128 DVE lanes, then DMA-broadcast each [64, 512]
                # operand back.
                scr_rec = dramp.tile(
                    [1, 1024], f32, tag="scr_rec", name=f"scrr_{b}_{half}"
                )
                packed = smallp.tile([32, 32], f32, tag="packed")
                nc.sync.dma_start(packed[:], den2[0:1, :])
                recp = smallp.tile([32, 32], f32, tag="recp")
                nc.vector.reciprocal(recp[:], packed[:])
                nc.sync.dma_start(
                    scr_rec[0, :].rearrange("(p f) -> p f", p=32), recp[:]
                )
                for h in range(HPC):
                    hp = slice(h * 64, (h + 1) * 64)
                    rec2 = smallp.tile(
                        [64, 512], f32, tag="rec2", name=f"rec2_{b}_{half}_{h}"
                    )
                    nc.sync.dma_start(
                        rec2[:],
                        _bass.AP(
                            scr_rec[:].tensor,
                            scr_rec[:].offset + 512 * h,
                            [[0, 64], [1, 512]],
                        ),
                    )
                    nc.vector.tensor_mul(
                        outT2[hp, t0:t0 + 512], po_h[h][0:64, 0:512], rec2[:]
                    )

            PROJ_COPY_ENG = ("v", "s", "v", "v")

            _fctr = [0]

            def emit_proj_tile(pb, o2, i, tt, wide=False):
                ob = poutp.tile([128, C], bf, tag="ob")
                for ct in range(2):
                    # During the flush (wide=True) the scores/transpose pools
                    # are drained, so cycle all three for a 6-bank rotation.
                    if wide:
                        _fctr[0] += 1
                        pool, tag = [(psproj, "psproj"), (ps512, "ps512"),
                                     (psvt, "psvt")][_fctr[0] % 3]
                        pp = pool.tile([128, 512], f32, tag=tag,
                                       name=f"fl{_fctr[0]}")
                    else:
                        pp = psproj.tile([128, 512], f32, tag="psproj")
                    nc.tensor.matmul(
                        pp[:],
                        o2[:, tt * 128:(tt + 1) * 128],
                        wp_s[:, ct * 512:(ct + 1) * 512],
                        start=True,
                        stop=True,
                    )
                    if wide:
                        eng = ("v", "s")[(2 * i + ct) % 2]
                    else:
                        eng = PROJ_COPY_ENG[(2 * i + ct) % 4]
                    dst = ob[:, ct * 512:(ct + 1) * 512]
                    if eng == "s":
                        nc.scalar.copy(dst, pp[:])
                    else:
                        nc.vector.tensor_copy(dst, pp[:])
                nc.sync.dma_start(
                    out_d[pb * T + tt * 128:pb * T + (tt + 1) * 128, :], ob[:]
                )

            def emit_proj_half(pb, o2, half, wide=False):
                for i, tt in enumerate(range(half * 4, half * 4 + 4)):
                    emit_proj_tile(pb, o2, i, tt, wide=wide)

            # ---- main pipeline ----
            pend = []  # pending proj halves: (batch, outT2, half)

            def pop_proj(b):
                # lag-2: only emit proj halves at least two batches old, so
                # the normalize DMA-bounce latency is always covered and the
                # tail flush has three proj halves of PE work in front of the
                # last (normalize-gated) one.  Returns a list of per-tile
                # thunks so callers can spread the tiles into the ACT-bound
                # scores loop as PE backfill.
                if pend and pend[0][0] <= b - 2:
                    pb, o2, half = pend.pop(0)
                    return [
                        (lambda i=i, tt=tt: emit_proj_tile(pb, o2, i, tt))
                        for i, tt in enumerate(range(half * 4, half * 4 + 4))
                    ]
                return []

            outs = {}
            for b in range(B):
                xb = xba[:, :, b * T:(b + 1) * T]
                vaug = vaugp.tile([128, NS, HPC, 66], bf, tag="vaug",
                                  name=f"vaug{b}")
                nc.vector.memset(vaug[:, :, :, 64:65], 1.0)
                outT2 = outtp.tile([128, T], bf, tag="outT2", name=f"outT2_{b}")
                outs[b] = outT2
                exs = {}

                qT, kT = emit_qk(b, xb)
                # scores s=0 early: gives ACT exp a head start over the
                # v/proj PE work that follows.
                emit_scores_s(b, 0, qT, kT, exs)
                vT = qkvp.tile([128, T], bf, tag="vT", name=f"vT{b}")
                emit_v_mms(b, xb, 0, vT)
                emit_scores_s(b, 1, qT, kT, exs)
                emit_v_mms(b, xb, 1, vT)
                emit_vtrans(b, vT, vaug, range(0, 4))
                slot_a = pop_proj(b)  # slot A: old proj half, spread out
                for t in slot_a[:2]:
                    t()
                emit_vtrans(b, vT, vaug, range(4, NS))

                po0 = [
                    psatt.tile([128, 512], f32, tag="psatt", name=f"po0_{b}_{h}")
                    for h in range(HPC)
                ]
                for s in range(2, NS):
                    emit_scores_s(b, s, qT, kT, exs)
                    if s - 2 <= 3:
                        emit_po0_s(b, s - 2, vaug, exs, po0)
                    if s - 2 == 3:
                        emit_normalize_half(b, 0, po0, outT2)
                    if s - 2 < len(slot_a) - 2:
                        slot_a[2 + (s - 2)]()  # PE backfill, no exp dep

                po1 = [
                    psatt.tile([128, 512], f32, tag="psatt", name=f"po1_{b}_{h}")
                    for h in range(HPC)
                ]
                for s in range(NS):
                    s0 = s * 128
                    d1 = max(0, s0 - 512)
                    for h in range(HPC):
                        nc.tensor.matmul(
                            po1[h][0:65, d1:512],
                            vaug[:, s, h, 0:65],
                            exs[(h, s)][:, 512 - s0 + d1:T - s0],
                            start=(s == 0),
                            stop=(s == NS - 1),
                        )
                pend.append((b, outT2, 0))
                emit_normalize_half(b, 1, po1, outT2)
                for t in pop_proj(b):  # slot B
                    t()
                pend.append((b, outT2, 1))

            while pend:
                pb, o2, half = pend.pop(0)
                emit_proj_half(pb, o2, half, wide=True)

    _split_multi_waits(nc, mybir)
    return nc


def _get_compiled():
    global _compiled
    if _compiled is None:
        _compiled = _build()
    return _compiled


def _shuf_w(W, h0):
    # [H, C, D] head-pair slice -> [C, D2] -> pre-shuffled [p, k, m] so the
    # device DMA is one contiguous [128, 1024] transfer per weight.
    w = np.asarray(W[h0:h0 + HPC], dtype=np.float32).transpose(1, 0, 2).reshape(C, D2)
    return np.ascontiguousarray(
        w.reshape(C // 128, 128, D2).transpose(1, 0, 2)
    ).astype(BF16)


def _make_in_maps(x, Wq, Wk, Wv, Wp):
    xT = np.ascontiguousarray(
        np.asarray(x, dtype=np.float32).reshape(BT, C).T
    ).astype(BF16)  # [C, BT]
    mask = np.triu(np.ones((128, 128), dtype=BF16))  # keep j >= i
    ident = np.eye(128, dtype=BF16)
    in_maps = []
    for i in range(NCORES):
        h0 = i * HPC
        wp = np.ascontiguousarray(
            np.asarray(Wp, dtype=np.float32)[h0 * D:(h0 + HPC) * D, :]
        ).astype(BF16)
        in_maps.append(
            {"xT": xT, "wq": _shuf_w(Wq, h0), "wk": _shuf_w(Wk, h0),
             "wv": _shuf_w(Wv, h0), "wp": wp, "mask": mask, "ident": ident}
        )
    return in_maps


def run(x, Wq, Wk, Wv, Wp, bp, trace=False, trace_cores=None):
    """Returns (full_output [B,T,C], BassKernelResults)."""
    from concourse.bass_utils import run_bass_kernel_spmd

    nc = _get_compiled()
    in_maps = _make_in_maps(x, Wq, Wk, Wv, Wp)
    kw = {}
    if trace:
        kw = {"trace": True, "trace_cores": trace_cores or [0]}
    res = run_bass_kernel_spmd(nc, in_maps, list(range(NCORES)), **kw)
    acc = np.zeros((BT, C), dtype=np.float32)
    for i in range(NCORES):
        acc += np.asarray(res.results[i]["out"], dtype=np.float32)
    acc += np.asarray(bp, dtype=np.float32)[None, :]
    return acc.reshape(B, T, C), res


def kernel(x, Wq, Wk, Wv, Wp, bp):
    out, _ = run(x, Wq, Wk, Wv, Wp, bp)
    return out
